# revision 23
# baseline (speedup 1.0000x reference)
"""GraphSAGE 2-layer forward on 8 Trainium2 NeuronCores (Bass/Tile).

Strategy (graph/data parallel, per sharding hint):
  - Destination nodes partitioned across 8 cores (12500 each); edges bucketed
    to the core owning their dst, grouped into 128-dst-node blocks, and blocks
    into groups of gb=4 (512 dsts) for coarse-grained pipelining.
  - Per-edge payloads are fetched with dma_gather in prepare_only mode +
    trigger_dma, so the Pool engine only spends descriptor-generation time and
    the SDMA transfers overlap with desc-gen of the next group and all compute
    (the v1 kernel's blocking gathers serialized the whole kernel).
  - 256B tokens (the SWDGE minimum) with 2 packed nodes per token:
      L0 table y0 = x @ Wl0.T as [50000, 128] fp16 (node pairs),
      L1 table y1 = h @ Wl1.T as [50000, 64] fp32 (2x16 values + 32 pad).
    int16 gather indices only span 32768 rows, so edges are split into a
    lo stream (src < 65536) and a hi stream with a rebased table AP.
  - A per-token mask*1/deg vector zeroes the wrong packed node and folds in
    the mean normalization; segment-sum over dst is one-hot matmul on PE:
    S[k, j] = (dst_rel[k] == j), psum[pair-feat, dst] += g.T @ S, pairs folded
    with a stacked-identity matmul.
  - Weights replicated; dense matmuls keep features on partitions; e0 is
    re-added per group via an identity matmul so everything lands in psum.
"""
import numpy as np
from contextlib import ExitStack

from concourse import bass, bacc, mybir, tile
from concourse.bass_utils import run_bass_kernel_spmd

dt = mybir.dt
PREP_TRIGGER = False   # prepare_only+trigger pipelining (tile sem gaps; off)
KMAX = 1024            # max idxs per blocking dma_gather call

# problem constants (hardcoded per harness contract)
N = 100000
FEAT = 128
EMB = 64
HID = 64
OUT = 16
NC_N = 8
NODES_PER = N // NC_N          # 12500
P = 128
BLOCKS = (NODES_PER + P - 1) // P   # 98
GB = 4                          # blocks per group
NGROUPS = (BLOCKS + GB - 1) // GB   # 25 (24x4 + 1x2)
SPLIT = 65536                   # lo/hi src stream split (int16 idx range)


def default_cfg(slots_a, slots_b):
    return dict(slots_a=slots_a, slots_b=slots_b,
                ch_a=slots_a // P, ch_b=slots_b // P)


def build_nc(cfg):
    sA, sB = cfg["slots_a"], cfg["slots_b"]
    chA, chB = cfg["ch_a"], cfg["ch_b"]
    chT = chA + chB
    spb = sA + sB                    # slots (tokens) per block
    ntok = BLOCKS * spb              # gather tokens per core
    nchunks = BLOCKS * chT
    tail = NODES_PER - (BLOCKS - 1) * P

    nc = bacc.Bacc("TRN2", target_bir_lowering=False, debug=False,
                   num_devices=NC_N)

    def din(name, shape, d):
        return nc.dram_tensor(name, shape, d, kind="ExternalInput").ap()

    # --- inputs (per core) ---
    xT = din("xT", [FEAT, NODES_PER], dt.float32)
    xembT = din("xembT", [2 * EMB, NODES_PER], dt.float32)
    srcp = din("srcp", [P, ntok // 16], dt.int16)
    dstrel = din("dstrel", [P, nchunks], dt.int16)
    maskrd = din("maskrd", [P, nchunks * 2], dt.float16)
    Wl0T = din("Wl0T", [FEAT, HID], dt.float32)
    Wr0T = din("Wr0T", [FEAT, HID], dt.float32)
    We0Th = din("We0Th", [EMB, HID], dt.float32)     # We0.T * 0.5
    Wr1T = din("Wr1T", [HID, OUT], dt.float16)
    We1T = din("We1T", [HID, OUT], dt.float16)
    Wl1T = din("Wl1T", [HID, OUT], dt.float16)
    b0 = din("b0", [HID, 1], dt.float32)             # bl0
    be0 = din("be0", [HID, 1], dt.float32)
    b1 = din("b1", [OUT, 1], dt.float32)             # bl1 + be1
    i642 = din("i642", [P, HID], dt.float16)         # vstack(I64, I64)
    i162 = din("i162", [2 * OUT, OUT], dt.float16)   # vstack(I16, I16)
    i64f = din("i64f", [EMB, HID], dt.float16)       # I64 fp16
    out_own = nc.dram_tensor("out_own", [NODES_PER, OUT], dt.float32,
                             kind="ExternalOutput").ap()

    EL0 = 2 * HID                  # 128 fp16 = 256B per token (2 nodes)
    EL1 = 4 * OUT                  # 64 fp32 = 256B per token (2 nodes + pad)
    gcolsmax = GB * chT            # chunk capacity of the per-group g tile

    with tile.TileContext(nc) as tc, ExitStack() as ctx:
        const = ctx.enter_context(tc.tile_pool(name="const", bufs=1))
        sb = ctx.enter_context(tc.tile_pool(name="sb", bufs=2))
        sb3 = ctx.enter_context(tc.tile_pool(name="sb3", bufs=2))
        ppA = ctx.enter_context(tc.tile_pool(name="ppA", bufs=2, space="PSUM"))
        ppB = ctx.enter_context(tc.tile_pool(name="ppB", bufs=2, space="PSUM"))
        ppC = ctx.enter_context(tc.tile_pool(name="ppC", bufs=2, space="PSUM"))
        dram = ctx.enter_context(tc.tile_pool(name="dram", bufs=1, space="DRAM"))

        # --- resident constants ---
        srcp_t = const.tile([P, ntok // 16], dt.int16)
        nc.sync.dma_start(srcp_t[:], srcp[:])
        dstrel_t = const.tile([P, nchunks], dt.int16)
        nc.sync.dma_start(dstrel_t[:], dstrel[:])
        maskrd_t = const.tile([P, nchunks * 2], dt.float16)
        nc.sync.dma_start(maskrd_t[:], maskrd[:])
        iota_t = const.tile([P, max(chA, chB) * P], dt.int16)
        nc.gpsimd.iota(iota_t[:], pattern=[[0, max(chA, chB)], [1, P]],
                       base=0, channel_multiplier=0)
        Wl0T_t = const.tile([FEAT, HID], dt.float32)
        nc.sync.dma_start(Wl0T_t[:], Wl0T[:])
        Wr0T_t = const.tile([FEAT, HID], dt.float32)
        nc.sync.dma_start(Wr0T_t[:], Wr0T[:])
        We0Th_t = const.tile([EMB, HID], dt.float32)
        nc.sync.dma_start(We0Th_t[:], We0Th[:])
        Wr1T_t = const.tile([HID, OUT], dt.float16)
        nc.sync.dma_start(Wr1T_t[:], Wr1T[:])
        We1T_t = const.tile([HID, OUT], dt.float16)
        nc.sync.dma_start(We1T_t[:], We1T[:])
        Wl1T_t = const.tile([HID, OUT], dt.float16)
        nc.sync.dma_start(Wl1T_t[:], Wl1T[:])
        b0_t = const.tile([HID, 1], dt.float32)
        nc.sync.dma_start(b0_t[:], b0[:])
        be0_t = const.tile([HID, 1], dt.float32)
        nc.sync.dma_start(be0_t[:], be0[:])
        b1_t = const.tile([OUT, 1], dt.float32)
        nc.sync.dma_start(b1_t[:], b1[:])
        i642_t = const.tile([P, HID], dt.float16)
        nc.sync.dma_start(i642_t[:], i642[:])
        i162_t = const.tile([2 * OUT, OUT], dt.float16)
        nc.sync.dma_start(i162_t[:], i162[:])
        i64f_t = const.tile([EMB, HID], dt.float16)
        nc.sync.dma_start(i64f_t[:], i64f[:])
        ident = const.tile([P, P], dt.float32)
        from concourse.masks import make_identity
        make_identity(nc, ident[:])
        zz32 = const.tile([P, 2 * OUT], dt.float32)
        nc.gpsimd.memset(zz32[:], 0.0)

        # persistent activations (features on partitions)
        e0T_t = const.tile([EMB, NODES_PER], dt.float16)
        hT_t = const.tile([HID, NODES_PER], dt.float16)

        dma_sem = nc.alloc_semaphore("swdge_dma")

        # =========== Phase 2: y0 = x @ Wl0.T (own nodes, pair-major fp16) ====
        y0_own = dram.tile([NODES_PER // 2, EL0], dt.float16)
        G = 512
        for g0 in range(0, NODES_PER, G):
            g1 = min(g0 + G, NODES_PER)
            w = g1 - g0
            xg = sb.tile([FEAT, G], dt.float32, tag="xg")
            nc.sync.dma_start(xg[:, :w], xT[:, g0:g1])
            py0 = ppB.tile([HID, G], dt.float32, tag="lin")
            nc.tensor.matmul(py0[:, :w], lhsT=Wl0T_t[:], rhs=xg[:, :w],
                             start=True, stop=True)
            y0g = sb.tile([HID, G], dt.float32, tag="y0g")
            nc.vector.tensor_copy(y0g[:, :w], py0[:, :w])
            for b0_ in range(0, w, P):
                b1_ = min(b0_ + P, w)
                bw = b1_ - b0_
                ptr = ppC.tile([P, HID], dt.float32, tag="tr")
                nc.tensor.transpose(ptr[:bw, :], y0g[:, b0_:b1_],
                                    ident[:HID, :HID])
                str_ = sb.tile([P, HID], dt.float16, tag="str")
                nc.vector.tensor_copy(str_[:bw, :], ptr[:bw, :])
                r0 = (g0 + b0_) // 2
                nc.sync.dma_start(
                    y0_own[r0:r0 + bw // 2, :].rearrange(
                        "r (a f) -> r a f", a=2),
                    str_[:bw, :])

        # =========== Phase 3: AllGather y0 -> table [50000, 128] f16 =========
        y0_full = dram.tile([N // 2, EL0], dt.float16, addr_space="Shared")
        nc.gpsimd.collective_compute(
            "AllGather", mybir.AluOpType.bypass,
            replica_groups=[list(range(NC_N))],
            ins=[y0_own[:]], outs=[y0_full[:]],
        )
        y0_lo = y0_full[0:SPLIT // 2, :]
        y0_hi = y0_full[SPLIT // 2:N // 2, :]

        # =========== Phase 1: e0T = (emb0+emb1) @ (0.5 We0.T) + be0 ==========
        # (after the AllGather kick so it overlaps the collective)
        for g0 in range(0, NODES_PER, G):
            g1 = min(g0 + G, NODES_PER)
            w = g1 - g0
            xe0 = sb.tile([EMB, G], dt.float32, tag="xe0")
            nc.sync.dma_start(xe0[:, :w], xembT[:EMB, g0:g1])
            xe1 = sb.tile([EMB, G], dt.float32, tag="xe1")
            nc.sync.dma_start(xe1[:, :w], xembT[EMB:, g0:g1])
            pe0 = ppB.tile([HID, G], dt.float32, tag="lin")
            nc.tensor.matmul(pe0[:, :w], lhsT=We0Th_t[:], rhs=xe0[:, :w],
                             start=True, stop=False)
            nc.tensor.matmul(pe0[:, :w], lhsT=We0Th_t[:], rhs=xe1[:, :w],
                             start=False, stop=True)
            nc.scalar.activation(e0T_t[:, g0:g1], pe0[:, :w],
                                 mybir.ActivationFunctionType.Identity,
                                 bias=be0_t[:])

        # =========== Phase 4: L0 aggregation + h assembly + y1 ===============
        y1_own = dram.tile([NODES_PER // 2, EL1], dt.float32)

        def groups():
            for gi in range(NGROUPS):
                blo = gi * GB
                bhi = min(blo + GB, BLOCKS)
                yield gi, blo, bhi, bhi - blo

        def gather_group(blo, nb, tab_lo, tab_hi, elem):
            """Issue lo+hi stream preps + trigger for blocks [blo, blo+nb)."""
            ctot = nb * chT
            g = sb3.tile([P, gcolsmax * EL0], dt.float16, tag="g")
            gv = g[:] if elem == EL0 else g[:].bitcast(dt.float32)
            t0 = blo * spb
            ga = nb * sA                      # lo-stream tokens in group
            gt = nb * spb
            if PREP_TRIGGER:
                nc.gpsimd.dma_gather(
                    out_ap=gv[:, 0:nb * chA * elem].rearrange(
                        "p (c e) -> p c e", e=elem),
                    in_ap=tab_lo,
                    idxs_ap=srcp_t[:, t0 // 16:(t0 + ga) // 16],
                    num_idxs=ga, num_idxs_reg=ga, elem_size=elem,
                    prepare_only=True, sem=dma_sem,
                )
                nc.gpsimd.dma_gather(
                    out_ap=gv[:, nb * chA * elem:ctot * elem].rearrange(
                        "p (c e) -> p c e", e=elem),
                    in_ap=tab_hi,
                    idxs_ap=srcp_t[:, (t0 + ga) // 16:(t0 + gt) // 16],
                    num_idxs=nb * sB, num_idxs_reg=nb * sB, elem_size=elem,
                    prepare_only=True, sem=dma_sem,
                )
                nc.gpsimd.trigger_dma(count=None)
            else:
                for base, n, tab, cof in ((t0, ga, tab_lo, 0),
                                          (t0 + ga, nb * sB, tab_hi,
                                           nb * chA)):
                    for k0 in range(0, n, KMAX):
                        k1 = min(k0 + KMAX, n)
                        nc.gpsimd.dma_gather(
                            out_ap=gv[:, (cof * P + k0) * elem // P:
                                      (cof * P + k1) * elem // P].rearrange(
                                "p (c e) -> p c e", e=elem),
                            in_ap=tab,
                            idxs_ap=srcp_t[:, (base + k0) // 16:
                                           (base + k1) // 16],
                            num_idxs=k1 - k0, num_idxs_reg=k1 - k0,
                            elem_size=elem,
                        )
            return g, gv

        def build_S(blo, nb):
            """Per-block one-hot S tiles for the group (A then B chunks)."""
            ctot = nb * chT
            c0 = blo * chT
            S = sb3.tile([P, gcolsmax * P], dt.float16, tag="S")
            for lb in range(nb):
                for (cr, cn, cglob) in (
                    (lb * chA, chA, c0 + lb * chA),
                    (nb * chA + lb * chB, chB, c0 + nb * chA + lb * chB),
                ):
                    dr = dstrel_t[:, cglob:cglob + cn]
                    nc.vector.tensor_tensor(
                        out=S[:, cr * P:(cr + cn) * P].rearrange(
                            "p (c q) -> p c q", q=P),
                        in0=iota_t[:, :cn * P].rearrange(
                            "p (c q) -> p c q", q=P),
                        in1=dr.unsqueeze(2).broadcast_to([P, cn, P]),
                        op=mybir.AluOpType.is_equal)
            return S

        def block_chunks(lb, nb):
            """(col, n) chunk runs of block lb within the group g/S tiles."""
            return ((lb * chA, chA), (nb * chA + lb * chB, chB))

        for gi, blo, bhi, nb in groups():
            ctot = nb * chT
            c0 = blo * chT
            nw = (bhi - blo - 1) * P + (tail if bhi == BLOCKS else P)
            g, _ = gather_group(blo, nb, y0_lo, y0_hi, EL0)
            # mask*1/deg zeroes the wrong node of each pair
            g4 = g[:, :ctot * EL0].rearrange("p (q f) -> p q f", f=HID)
            mr = maskrd_t[:, c0 * 2:(c0 + ctot) * 2]
            nc.vector.tensor_tensor(
                out=g4, in0=g4,
                in1=mr.unsqueeze(2).broadcast_to([P, ctot * 2, HID]),
                op=mybir.AluOpType.mult)
            S = build_S(blo, nb)
            pag = ppA.tile([P, GB * P], dt.float32, tag="pag")
            for lb in range(nb):
                bw = tail if blo + lb == BLOCKS - 1 else P
                runs = block_chunks(lb, nb)
                last_cr = runs[-1][0] + runs[-1][1] - 1
                for cr, cn in runs:
                    for j in range(cn):
                        cc = cr + j
                        nc.tensor.matmul(
                            pag[:, lb * P:lb * P + bw],
                            lhsT=g[:, cc * EL0:(cc + 1) * EL0],
                            rhs=S[:, cc * P:cc * P + bw],
                            start=(cc == runs[0][0]), stop=(cc == last_cr))
            aggP = sb.tile([P, GB * P], dt.float16, tag="aggP")
            nc.vector.tensor_copy(aggP[:, :nw], pag[:, :nw])
            xb = sb.tile([FEAT, GB * P], dt.float32, tag="xg")
            nc.sync.dma_start(xb[:, :nw], xT[:, blo * P:blo * P + nw])
            ph = ppB.tile([HID, GB * P], dt.float32, tag="lin")
            nc.tensor.matmul(ph[:, :nw], lhsT=Wr0T_t[:], rhs=xb[:, :nw],
                             start=True, stop=False)
            nc.tensor.matmul(ph[:, :nw], lhsT=i64f_t[:],
                             rhs=e0T_t[:, blo * P:blo * P + nw],
                             start=False, stop=False)
            nc.tensor.matmul(ph[:, :nw], lhsT=i642_t[:], rhs=aggP[:, :nw],
                             start=False, stop=True)
            # h = relu(agg + xWr0 + e0 + bl0)
            nc.scalar.activation(hT_t[:, blo * P:blo * P + nw], ph[:, :nw],
                                 mybir.ActivationFunctionType.Relu,
                                 bias=b0_t[:])
            # y1 = h @ Wl1.T -> pair-major fp32 (+zero pad cols)
            py1 = ppB.tile([HID, GB * P], dt.float32, tag="lin")
            nc.tensor.matmul(py1[:OUT, :nw], lhsT=Wl1T_t[:],
                             rhs=hT_t[:, blo * P:blo * P + nw],
                             start=True, stop=True)
            y1sb = sb.tile([OUT, GB * P], dt.float32, tag="o16")
            nc.vector.tensor_copy(y1sb[:, :nw], py1[:OUT, :nw])
            for b0_ in range(0, nw, P):
                bw = min(P, nw - b0_)
                ptr1 = ppC.tile([P, HID], dt.float32, tag="tr")
                nc.tensor.transpose(ptr1[:bw, :OUT], y1sb[:, b0_:b0_ + bw],
                                    ident[:OUT, :OUT])
                y1tr = sb.tile([P, OUT], dt.float32, tag="y1tr")
                nc.vector.tensor_copy(y1tr[:bw, :], ptr1[:bw, :OUT])
                r0 = (blo * P + b0_) // 2
                nc.sync.dma_start(
                    y1_own[r0:r0 + bw // 2, 0:2 * OUT].rearrange(
                        "r (a f) -> r a f", a=2),
                    y1tr[:bw, :])
                nc.sync.dma_start(
                    y1_own[r0:r0 + bw // 2, 2 * OUT:4 * OUT].rearrange(
                        "r (a f) -> r a f", a=2),
                    zz32[:bw, :OUT])

        # =========== Phase 5: AllGather y1 ===================================
        y1_full = dram.tile([N // 2, EL1], dt.float32, addr_space="Shared")
        nc.gpsimd.collective_compute(
            "AllGather", mybir.AluOpType.bypass,
            replica_groups=[list(range(NC_N))],
            ins=[y1_own[:]], outs=[y1_full[:]],
        )
        y1_lo = y1_full[0:SPLIT // 2, :]
        y1_hi = y1_full[SPLIT // 2:N // 2, :]

        # =========== Phase 6: L1 aggregation + output ========================
        for gi, blo, bhi, nb in groups():
            ctot = nb * chT
            c0 = blo * chT
            nw = (bhi - blo - 1) * P + (tail if bhi == BLOCKS else P)
            g, gv = gather_group(blo, nb, y1_lo, y1_hi, EL1)
            # masked compaction: keep the 2 real 16-wide rows of each token
            gm = sb3.tile([P, gcolsmax * 2 * OUT], dt.float16, tag="gm")
            for q in range(2):
                nc.vector.tensor_tensor(
                    out=gm[:, :ctot * 2 * OUT].rearrange(
                        "p (c q2 f) -> p c q2 f", q2=2, f=OUT)
                        [:, :, q:q + 1, :],
                    in0=gv[:, :ctot * EL1].rearrange(
                        "p (c q4 f) -> p c q4 f", q4=4, f=OUT)
                        [:, :, q:q + 1, :],
                    in1=maskrd_t[:, c0 * 2:(c0 + ctot) * 2].rearrange(
                        "p (c q2) -> p c q2", q2=2)[:, :, q:q + 1]
                        .unsqueeze(3).broadcast_to([P, ctot, 1, OUT]),
                    op=mybir.AluOpType.mult)
            S = build_S(blo, nb)
            pag = ppA.tile([P, GB * P], dt.float32, tag="pag")
            for lb in range(nb):
                bw = tail if blo + lb == BLOCKS - 1 else P
                runs = block_chunks(lb, nb)
                last_cr = runs[-1][0] + runs[-1][1] - 1
                for cr, cn in runs:
                    for j in range(cn):
                        cc = cr + j
                        nc.tensor.matmul(
                            pag[:2 * OUT, lb * P:lb * P + bw],
                            lhsT=gm[:, cc * 2 * OUT:(cc + 1) * 2 * OUT],
                            rhs=S[:, cc * P:cc * P + bw],
                            start=(cc == runs[0][0]), stop=(cc == last_cr))
            aggP1 = sb.tile([2 * OUT, GB * P], dt.float16, tag="aggP1")
            nc.vector.tensor_copy(aggP1[:, :nw], pag[:2 * OUT, :nw])
            ph1 = ppB.tile([HID, GB * P], dt.float32, tag="lin")
            nc.tensor.matmul(ph1[:OUT, :nw], lhsT=i162_t[:],
                             rhs=aggP1[:, :nw], start=True, stop=False)
            nc.tensor.matmul(ph1[:OUT, :nw], lhsT=Wr1T_t[:],
                             rhs=hT_t[:, blo * P:blo * P + nw],
                             start=False, stop=False)
            nc.tensor.matmul(ph1[:OUT, :nw], lhsT=We1T_t[:],
                             rhs=e0T_t[:, blo * P:blo * P + nw],
                             start=False, stop=True)
            oT = sb.tile([OUT, GB * P], dt.float32, tag="o16")
            nc.scalar.activation(oT[:, :nw], ph1[:OUT, :nw],
                                 mybir.ActivationFunctionType.Identity,
                                 bias=b1_t[:])
            for b0_ in range(0, nw, P):
                bw = min(P, nw - b0_)
                ptro = ppC.tile([P, HID], dt.float32, tag="tr")
                nc.tensor.transpose(ptro[:bw, :OUT], oT[:, b0_:b0_ + bw],
                                    ident[:OUT, :OUT])
                osb = sb.tile([P, OUT], dt.float32, tag="osb")
                nc.vector.tensor_copy(osb[:bw, :], ptro[:bw, :OUT])
                nc.sync.dma_start(
                    out_own[blo * P + b0_:blo * P + b0_ + bw, :],
                    osb[:bw, :])

    # --- fix up prepare_only gather completion sems -----------------------
    # Tile assigns each Pool-DMA prep a DMASW{lane} proc and makes data
    # consumers wait on that lane's semaphore, but the semaphore actually
    # baked into the descriptors (fired by SDMA at transfer completion) is
    # the caller's sem= argument. Rewrite each prep's OnUpdate[0] to the
    # scheduler's DMASW lane semaphore so the consumer waits are satisfied
    # by the real DMA completion.
    if PREP_TRIGGER:
        from concourse.tile_scheduler import dmasw_start_idx
        lane_sems = {}
        for bb in nc.m.functions[0].blocks:
            for ins in bb.instructions:
                si = getattr(ins, "sync_info", None)
                if si is None:
                    continue
                for w in si.on_wait:
                    nm = w.ant_name or ""
                    if nm.startswith("DMASW"):
                        lane_sems[int(nm[5:nm.index("_")])] = (w.id, nm)
        for bb in nc.m.functions[0].blocks:
            for ins in bb.instructions:
                if (type(ins).__name__ == "InstDMAGatherAnt"
                        and getattr(ins, "gen_mode", 0) == 1):
                    lane = ins.bass_scheduled_proc - dmasw_start_idx
                    sid, snm = lane_sems[lane]
                    u = ins.sync_info.on_update[0]
                    assert u.ant_name == "swdge_dma", u.ant_name
                    u.id = sid
                    u.ant_name = snm

    nc.compile()
    return nc


def prep_inputs(cfg, x_feat, x_emb, edge_index,
                Wl0, bl0, Wr0, We0, be0, Wl1, bl1, Wr1, We1, be1):
    """Shard + layout inputs for each core. Pure indexing/layout (plus weight
    transposes and the bias-constant folds); all FLOPs stay on device."""
    sA, sB = cfg["slots_a"], cfg["slots_b"]
    chT = cfg["ch_a"] + cfg["ch_b"]
    spb = sA + sB
    ntok = BLOCKS * spb
    nchunks = BLOCKS * chT

    src, dst = np.asarray(edge_index[0]), np.asarray(edge_index[1])
    in_maps = []
    w_common = dict(
        Wl0T=np.ascontiguousarray(Wl0.T, np.float32),
        Wr0T=np.ascontiguousarray(Wr0.T, np.float32),
        We0Th=np.ascontiguousarray(We0.T * 0.5, np.float32),
        Wr1T=np.ascontiguousarray(Wr1.T).astype(np.float16),
        We1T=np.ascontiguousarray(We1.T).astype(np.float16),
        Wl1T=np.ascontiguousarray(Wl1.T).astype(np.float16),
        b0=np.asarray(bl0, np.float32).reshape(-1, 1),
        be0=np.asarray(be0, np.float32).reshape(-1, 1),
        b1=(np.asarray(bl1, np.float32) + np.asarray(be1, np.float32)
            ).reshape(-1, 1),
        i642=np.tile(np.eye(64, dtype=np.float16), (2, 1)),
        i162=np.tile(np.eye(16, dtype=np.float16), (2, 1)),
        i64f=np.eye(64, dtype=np.float16),
    )
    for c in range(NC_N):
        lo = c * NODES_PER
        m = (dst >= lo) & (dst < lo + NODES_PER)
        s_c, d_c = src[m], dst[m] - lo
        deg = np.bincount(d_c, minlength=NODES_PER)
        rdeg = 1.0 / np.maximum(deg, 1.0)
        blk = d_c >> 7
        strm = (s_c >= SPLIT).astype(np.int64)
        key = blk * 2 + strm
        order = np.argsort(key, kind="stable")
        s_c, d_c, blk, strm, key = (s_c[order], d_c[order], blk[order],
                                    strm[order], key[order])
        cnt = np.bincount(key, minlength=BLOCKS * 2)
        if cnt[0::2].max() > sA or cnt[1::2].max() > sB:
            raise OverflowError(int(cnt[0::2].max()), int(cnt[1::2].max()))
        starts = np.zeros(BLOCKS * 2, np.int64)
        starts[1:] = np.cumsum(cnt)[:-1]
        slot = np.arange(len(d_c)) - starts[key]
        gi = blk // GB
        blo = gi * GB
        nb = np.minimum(blo + GB, BLOCKS) - blo
        lb = blk - blo
        t = blo * spb + np.where(
            strm == 0,
            lb * sA + slot,
            nb * sA + lb * sB + slot)

        srcp = np.zeros(ntok, np.int16)
        srcp[t] = np.where(s_c < SPLIT, s_c >> 1,
                           (s_c - SPLIT) >> 1).astype(np.int16)
        dstrel = np.full((P, nchunks), -1, np.int16)
        dstrel[t % P, t // P] = (d_c & 127).astype(np.int16)
        maskrd = np.zeros((P, nchunks, 2), np.float16)
        maskrd[t % P, t // P, s_c & 1] = rdeg[d_c].astype(np.float16)

        in_maps.append(dict(
            xT=np.ascontiguousarray(
                np.asarray(x_feat[lo:lo + NODES_PER]).T, np.float32),
            xembT=np.ascontiguousarray(
                np.asarray(x_emb[lo:lo + NODES_PER])
                .transpose(1, 2, 0).reshape(2 * EMB, NODES_PER), np.float32),
            srcp=np.ascontiguousarray(np.tile(srcp.reshape(-1, 16).T, (8, 1))),
            dstrel=np.ascontiguousarray(dstrel),
            maskrd=np.ascontiguousarray(maskrd.reshape(P, -1)),
            **w_common,
        ))
    return in_maps


def kernel(**inputs):
    src_dst = np.asarray(inputs["edge_index"])
    src, dst = src_dst[0], src_dst[1]
    mxA = mxB = 0
    for c in range(NC_N):
        lo = c * NODES_PER
        m = (dst >= lo) & (dst < lo + NODES_PER)
        s_c, d_c = src[m], dst[m] - lo
        key = (d_c >> 7) * 2 + (s_c >= SPLIT)
        cnt = np.bincount(key, minlength=BLOCKS * 2)
        mxA = max(mxA, int(cnt[0::2].max()))
        mxB = max(mxB, int(cnt[1::2].max()))
    slots_a = max(((mxA + P - 1) // P) * P, P)
    slots_b = max(((mxB + P - 1) // P) * P, P)
    cfg = default_cfg(slots_a, slots_b)
    nc = build_nc(cfg)
    in_maps = prep_inputs(cfg, **inputs)
    res = run_bass_kernel_spmd(nc, in_maps, list(range(NC_N)))
    kernel.last_res = res
    return np.concatenate([res.results[c]["out_own"] for c in range(NC_N)],
                          axis=0)


# revision 34
# speedup vs baseline: 1.0395x; 1.0395x over previous
"""GraphSAGE 2-layer forward on 8 Trainium2 NeuronCores (Bass/Tile).

Strategy (graph/data parallel, per sharding hint):
  - Destination nodes partitioned across 8 cores (12500 each); edges bucketed
    to the core owning their dst, grouped into 128-dst-node blocks, and blocks
    into groups of gb=4 (512 dsts) for coarse-grained pipelining.
  - Per-edge payloads are fetched with dma_gather in prepare_only mode +
    trigger_dma, so the Pool engine only spends descriptor-generation time and
    the SDMA transfers overlap with desc-gen of the next group and all compute
    (the v1 kernel's blocking gathers serialized the whole kernel).
  - 256B tokens (the SWDGE minimum) with 2 packed nodes per token:
      L0 table y0 = x @ Wl0.T as [50000, 128] fp16 (node pairs),
      L1 table y1 = h @ Wl1.T as [50000, 64] fp32 (2x16 values + 32 pad).
    int16 gather indices only span 32768 rows, so edges are split into a
    lo stream (src < 65536) and a hi stream with a rebased table AP.
  - A per-token mask*1/deg vector zeroes the wrong packed node and folds in
    the mean normalization; segment-sum over dst is one-hot matmul on PE:
    S[k, j] = (dst_rel[k] == j), psum[pair-feat, dst] += g.T @ S, pairs folded
    with a stacked-identity matmul.
  - Weights replicated; dense matmuls keep features on partitions; e0 is
    re-added per group via an identity matmul so everything lands in psum.
"""
import numpy as np
from contextlib import ExitStack

from concourse import bass, bacc, mybir, tile
from concourse.bass_utils import run_bass_kernel_spmd

dt = mybir.dt
PREP_TRIGGER = False   # prepare_only+trigger corrupts data on HW; keep off
KMAX = 1024            # max idxs per dma_gather call (SWDGE ring capacity)
SINGLE_PACKET = False  # stream packets per engine instead of one big packet

# problem constants (hardcoded per harness contract)
N = 100000
FEAT = 128
EMB = 64
HID = 64
OUT = 16
NC_N = 8
NODES_PER = N // NC_N          # 12500
P = 128
BLOCKS = (NODES_PER + P - 1) // P   # 98
GB = 4                          # blocks per group
NGROUPS = (BLOCKS + GB - 1) // GB   # 25 (24x4 + 1x2)
SPLIT = 65536                   # lo/hi src stream split (int16 idx range)


def default_cfg(slots_a, slots_b):
    return dict(slots_a=slots_a, slots_b=slots_b,
                ch_a=slots_a // P, ch_b=slots_b // P)


def build_nc(cfg):
    sA, sB = cfg["slots_a"], cfg["slots_b"]
    chA, chB = cfg["ch_a"], cfg["ch_b"]
    chT = chA + chB
    spb = sA + sB                    # slots (tokens) per block
    ntok = BLOCKS * spb              # gather tokens per core
    nchunks = BLOCKS * chT
    tail = NODES_PER - (BLOCKS - 1) * P

    nc = bacc.Bacc("TRN2", target_bir_lowering=False, debug=False,
                   num_devices=NC_N, dynamic_dma_scratch_size=32768,
                   detect_race_conditions=False)

    def din(name, shape, d):
        return nc.dram_tensor(name, shape, d, kind="ExternalInput").ap()

    # --- inputs (per core) ---
    xT = din("xT", [FEAT, NODES_PER], dt.float32)
    xembT = din("xembT", [2 * EMB, NODES_PER], dt.float32)
    srcp = din("srcp", [P, ntok // 16], dt.int16)
    dstrel = din("dstrel", [P, nchunks], dt.int16)
    maskrd = din("maskrd", [P, nchunks * 2], dt.float16)
    Wl0T = din("Wl0T", [FEAT, HID], dt.float32)
    Wr0T = din("Wr0T", [FEAT, HID], dt.float32)
    We0Th = din("We0Th", [EMB, HID], dt.float32)     # We0.T * 0.5
    Wr1T = din("Wr1T", [HID, OUT], dt.float16)
    We1T = din("We1T", [HID, OUT], dt.float16)
    Wl1T = din("Wl1T", [HID, OUT], dt.float16)
    b0 = din("b0", [HID, 1], dt.float32)             # bl0
    be0 = din("be0", [HID, 1], dt.float32)
    b1 = din("b1", [OUT, 1], dt.float32)             # bl1 + be1
    i642 = din("i642", [P, HID], dt.float16)         # vstack(I64, I64)
    i162 = din("i162", [2 * OUT, OUT], dt.float16)   # vstack(I16, I16)
    i64f = din("i64f", [EMB, HID], dt.float16)       # I64 fp16
    out_own = nc.dram_tensor("out_own", [NODES_PER, OUT], dt.float32,
                             kind="ExternalOutput").ap()

    EL0 = 2 * HID                  # 128 fp16 = 256B per token (2 nodes)
    EL1 = 4 * OUT                  # 64 fp32 = 256B per token (2 nodes + pad)
    gcolsmax = GB * chT            # chunk capacity of the per-group g tile

    with tile.TileContext(nc) as tc, ExitStack() as ctx:
        const = ctx.enter_context(tc.tile_pool(name="const", bufs=1))
        sb = ctx.enter_context(tc.tile_pool(name="sb", bufs=2))
        sb3 = ctx.enter_context(tc.tile_pool(name="sb3", bufs=2))
        sbg = ctx.enter_context(tc.tile_pool(name="sbg", bufs=1))
        ppA = ctx.enter_context(tc.tile_pool(name="ppA", bufs=2, space="PSUM"))
        ppB = ctx.enter_context(tc.tile_pool(name="ppB", bufs=2, space="PSUM"))
        ppC = ctx.enter_context(tc.tile_pool(name="ppC", bufs=2, space="PSUM"))
        dram = ctx.enter_context(tc.tile_pool(name="dram", bufs=1, space="DRAM"))

        # --- resident constants ---
        srcp_t = const.tile([P, ntok // 16], dt.int16)
        nc.sync.dma_start(srcp_t[:], srcp[:])
        dstrel_t = const.tile([P, nchunks], dt.int16)
        nc.sync.dma_start(dstrel_t[:], dstrel[:])
        maskrd_t = const.tile([P, nchunks * 2], dt.float16)
        nc.sync.dma_start(maskrd_t[:], maskrd[:])
        iota_t = const.tile([P, max(chA, chB) * P], dt.int16)
        nc.gpsimd.iota(iota_t[:], pattern=[[0, max(chA, chB)], [1, P]],
                       base=0, channel_multiplier=0)
        Wl0T_t = const.tile([FEAT, HID], dt.float32)
        nc.sync.dma_start(Wl0T_t[:], Wl0T[:])
        Wr0T_t = const.tile([FEAT, HID], dt.float32)
        nc.sync.dma_start(Wr0T_t[:], Wr0T[:])
        We0Th_t = const.tile([EMB, HID], dt.float32)
        nc.sync.dma_start(We0Th_t[:], We0Th[:])
        Wr1T_t = const.tile([HID, OUT], dt.float16)
        nc.sync.dma_start(Wr1T_t[:], Wr1T[:])
        We1T_t = const.tile([HID, OUT], dt.float16)
        nc.sync.dma_start(We1T_t[:], We1T[:])
        Wl1T_t = const.tile([HID, OUT], dt.float16)
        nc.sync.dma_start(Wl1T_t[:], Wl1T[:])
        b0_t = const.tile([HID, 1], dt.float32)
        nc.sync.dma_start(b0_t[:], b0[:])
        be0_t = const.tile([HID, 1], dt.float32)
        nc.sync.dma_start(be0_t[:], be0[:])
        b1_t = const.tile([OUT, 1], dt.float32)
        nc.sync.dma_start(b1_t[:], b1[:])
        i642_t = const.tile([P, HID], dt.float16)
        nc.sync.dma_start(i642_t[:], i642[:])
        i162_t = const.tile([2 * OUT, OUT], dt.float16)
        nc.sync.dma_start(i162_t[:], i162[:])
        i64f_t = const.tile([EMB, HID], dt.float16)
        nc.sync.dma_start(i64f_t[:], i64f[:])
        ident = const.tile([P, P], dt.float32)
        from concourse.masks import make_identity
        make_identity(nc, ident[:])
        zz32 = const.tile([P, 2 * OUT], dt.float32)
        nc.gpsimd.memset(zz32[:], 0.0)

        # persistent activations (features on partitions)
        e0T_t = const.tile([EMB, NODES_PER], dt.float16)
        hT_t = const.tile([HID, NODES_PER], dt.float16)

        dma_sem = nc.alloc_semaphore("swdge_dma")

        # =========== Phase 2: y0 = x @ Wl0.T (own nodes, pair-major fp16) ====
        y0_own = dram.tile([NODES_PER // 2, EL0], dt.float16)
        G = 512
        for g0 in range(0, NODES_PER, G):
            g1 = min(g0 + G, NODES_PER)
            w = g1 - g0
            xg = sb.tile([FEAT, G], dt.float32, tag="xg")
            nc.sync.dma_start(xg[:, :w], xT[:, g0:g1])
            py0 = ppB.tile([HID, G], dt.float32, tag="lin")
            nc.tensor.matmul(py0[:, :w], lhsT=Wl0T_t[:], rhs=xg[:, :w],
                             start=True, stop=True)
            y0g = sb.tile([HID, G], dt.float32, tag="y0g")
            nc.vector.tensor_copy(y0g[:, :w], py0[:, :w])
            for b0_ in range(0, w, P):
                b1_ = min(b0_ + P, w)
                bw = b1_ - b0_
                ptr = ppC.tile([P, HID], dt.float32, tag="tr")
                nc.tensor.transpose(ptr[:bw, :], y0g[:, b0_:b1_],
                                    ident[:HID, :HID])
                str_ = sb.tile([P, HID], dt.float16, tag="str")
                nc.vector.tensor_copy(str_[:bw, :], ptr[:bw, :])
                r0 = (g0 + b0_) // 2
                nc.sync.dma_start(
                    y0_own[r0:r0 + bw // 2, :].rearrange(
                        "r (a f) -> r a f", a=2),
                    str_[:bw, :])

        # =========== Phase 3: AllGather y0 -> table [50000, 128] f16 =========
        y0_full = dram.tile([N // 2, EL0], dt.float16, addr_space="Shared")
        nc.gpsimd.collective_compute(
            "AllGather", mybir.AluOpType.bypass,
            replica_groups=[list(range(NC_N))],
            ins=[y0_own[:]], outs=[y0_full[:]],
        )
        y0_lo = y0_full[0:SPLIT // 2, :]
        y0_hi = y0_full[SPLIT // 2:N // 2, :]

        # =========== Phase 1: e0T = (emb0+emb1) @ (0.5 We0.T) + be0 ==========
        # (after the AllGather kick so it overlaps the collective)
        for g0 in range(0, NODES_PER, G):
            g1 = min(g0 + G, NODES_PER)
            w = g1 - g0
            xe0 = sb.tile([EMB, G], dt.float32, tag="xe0")
            nc.sync.dma_start(xe0[:, :w], xembT[:EMB, g0:g1])
            xe1 = sb.tile([EMB, G], dt.float32, tag="xe1")
            nc.sync.dma_start(xe1[:, :w], xembT[EMB:, g0:g1])
            pe0 = ppB.tile([HID, G], dt.float32, tag="lin")
            nc.tensor.matmul(pe0[:, :w], lhsT=We0Th_t[:], rhs=xe0[:, :w],
                             start=True, stop=False)
            nc.tensor.matmul(pe0[:, :w], lhsT=We0Th_t[:], rhs=xe1[:, :w],
                             start=False, stop=True)
            nc.scalar.activation(e0T_t[:, g0:g1], pe0[:, :w],
                                 mybir.ActivationFunctionType.Identity,
                                 bias=be0_t[:])

        # =========== Phase 4: L0 aggregation + h assembly + y1 ===============
        y1_own = dram.tile([NODES_PER // 2, EL1], dt.float32)

        def groups():
            for gi in range(NGROUPS):
                blo = gi * GB
                bhi = min(blo + GB, BLOCKS)
                yield gi, blo, bhi, bhi - blo

        def gather_group(blo, nb, tab_lo, tab_hi, elem):
            """Issue lo+hi stream preps + trigger for blocks [blo, blo+nb)."""
            ctot = nb * chT
            g = sb3.tile([P, gcolsmax * EL0], dt.float16, tag="g")
            gv = g[:] if elem == EL0 else g[:].bitcast(dt.float32)
            t0 = blo * spb
            ga = nb * sA                      # lo-stream tokens in group
            gt = nb * spb
            for base, n, tab, cof in ((t0, ga, tab_lo, 0),
                                      (t0 + ga, nb * sB, tab_hi,
                                       nb * chA)):
                for k0 in range(0, n, KMAX):
                    k1 = min(k0 + KMAX, n)
                    kw = dict(prepare_only=True, sem=dma_sem) \
                        if PREP_TRIGGER else {}
                    kw["single_packet"] = SINGLE_PACKET
                    nc.gpsimd.dma_gather(
                        out_ap=gv[:, (cof * P + k0) * elem // P:
                                  (cof * P + k1) * elem // P].rearrange(
                            "p (c e) -> p c e", e=elem),
                        in_ap=tab,
                        idxs_ap=srcp_t[:, (base + k0) // 16:
                                       (base + k1) // 16],
                        num_idxs=k1 - k0, num_idxs_reg=k1 - k0,
                        elem_size=elem, **kw,
                    )
                    if PREP_TRIGGER:
                        nc.gpsimd.trigger_dma(count=None)
            return g, gv

        def build_S_block(blo, lb, nb):
            """One-hot S for block lb of the group: A chunks then B chunks."""
            c0 = blo * chT
            S = sb3.tile([P, chT * P], dt.float16, tag="S")
            for (sr, cn, cglob) in (
                (0, chA, c0 + lb * chA),
                (chA, chB, c0 + nb * chA + lb * chB),
            ):
                dr = dstrel_t[:, cglob:cglob + cn]
                nc.vector.tensor_tensor(
                    out=S[:, sr * P:(sr + cn) * P].rearrange(
                        "p (c q) -> p c q", q=P),
                    in0=iota_t[:, :cn * P].rearrange(
                        "p (c q) -> p c q", q=P),
                    in1=dr.unsqueeze(2).broadcast_to([P, cn, P]),
                    op=mybir.AluOpType.is_equal)
            return S

        def block_chunks(lb, nb):
            """(g-col, S-col, n) chunk runs of block lb in the group tile."""
            return ((lb * chA, 0, chA), (nb * chA + lb * chB, chA, chB))

        for gi, blo, bhi, nb in groups():
            ctot = nb * chT
            c0 = blo * chT
            nw = (bhi - blo - 1) * P + (tail if bhi == BLOCKS else P)
            g, _ = gather_group(blo, nb, y0_lo, y0_hi, EL0)
            # mask*1/deg zeroes the wrong node of each pair
            g4 = g[:, :ctot * EL0].rearrange("p (q f) -> p q f", f=HID)
            mr = maskrd_t[:, c0 * 2:(c0 + ctot) * 2]
            nc.vector.tensor_tensor(
                out=g4, in0=g4,
                in1=mr.unsqueeze(2).broadcast_to([P, ctot * 2, HID]),
                op=mybir.AluOpType.mult)
            pag = ppA.tile([P, GB * P], dt.float32, tag="pag")
            for lb in range(nb):
                bw = tail if blo + lb == BLOCKS - 1 else P
                S = build_S_block(blo, lb, nb)
                for cr, sr, cn in block_chunks(lb, nb):
                    for j in range(cn):
                        nc.tensor.matmul(
                            pag[:, lb * P:lb * P + bw],
                            lhsT=g[:, (cr + j) * EL0:(cr + j + 1) * EL0],
                            rhs=S[:, (sr + j) * P:(sr + j) * P + bw],
                            start=(sr + j == 0),
                            stop=(sr + j == chT - 1))
            aggP = sb.tile([P, GB * P], dt.float16, tag="aggP")
            nc.vector.tensor_copy(aggP[:, :nw], pag[:, :nw])
            xb = sb.tile([FEAT, GB * P], dt.float32, tag="xg")
            nc.sync.dma_start(xb[:, :nw], xT[:, blo * P:blo * P + nw])
            ph = ppB.tile([HID, GB * P], dt.float32, tag="lin")
            nc.tensor.matmul(ph[:, :nw], lhsT=Wr0T_t[:], rhs=xb[:, :nw],
                             start=True, stop=False)
            nc.tensor.matmul(ph[:, :nw], lhsT=i64f_t[:],
                             rhs=e0T_t[:, blo * P:blo * P + nw],
                             start=False, stop=False)
            nc.tensor.matmul(ph[:, :nw], lhsT=i642_t[:], rhs=aggP[:, :nw],
                             start=False, stop=True)
            # h = relu(agg + xWr0 + e0 + bl0)
            nc.scalar.activation(hT_t[:, blo * P:blo * P + nw], ph[:, :nw],
                                 mybir.ActivationFunctionType.Relu,
                                 bias=b0_t[:])
            # y1 = h @ Wl1.T -> pair-major fp32 (+zero pad cols)
            py1 = ppB.tile([HID, GB * P], dt.float32, tag="lin")
            nc.tensor.matmul(py1[:OUT, :nw], lhsT=Wl1T_t[:],
                             rhs=hT_t[:, blo * P:blo * P + nw],
                             start=True, stop=True)
            y1sb = sb.tile([OUT, GB * P], dt.float32, tag="o16")
            nc.vector.tensor_copy(y1sb[:, :nw], py1[:OUT, :nw])
            for b0_ in range(0, nw, P):
                bw = min(P, nw - b0_)
                ptr1 = ppC.tile([P, HID], dt.float32, tag="tr")
                nc.tensor.transpose(ptr1[:bw, :OUT], y1sb[:, b0_:b0_ + bw],
                                    ident[:OUT, :OUT])
                y1tr = sb.tile([P, OUT], dt.float32, tag="y1tr")
                nc.vector.tensor_copy(y1tr[:bw, :], ptr1[:bw, :OUT])
                r0 = (blo * P + b0_) // 2
                nc.sync.dma_start(
                    y1_own[r0:r0 + bw // 2, 0:2 * OUT].rearrange(
                        "r (a f) -> r a f", a=2),
                    y1tr[:bw, :])
                nc.sync.dma_start(
                    y1_own[r0:r0 + bw // 2, 2 * OUT:4 * OUT].rearrange(
                        "r (a f) -> r a f", a=2),
                    zz32[:bw, :OUT])

        # =========== Phase 5: AllGather y1 ===================================
        y1_full = dram.tile([N // 2, EL1], dt.float32, addr_space="Shared")
        nc.gpsimd.collective_compute(
            "AllGather", mybir.AluOpType.bypass,
            replica_groups=[list(range(NC_N))],
            ins=[y1_own[:]], outs=[y1_full[:]],
        )
        y1_lo = y1_full[0:SPLIT // 2, :]
        y1_hi = y1_full[SPLIT // 2:N // 2, :]

        # =========== Phase 6: L1 aggregation + output ========================
        for gi, blo, bhi, nb in groups():
            ctot = nb * chT
            c0 = blo * chT
            nw = (bhi - blo - 1) * P + (tail if bhi == BLOCKS else P)
            g, gv = gather_group(blo, nb, y1_lo, y1_hi, EL1)
            # masked compaction: keep the 2 real 16-wide rows of each token
            gm = sbg.tile([P, gcolsmax * 2 * OUT], dt.float16, tag="gm")
            for q in range(2):
                nc.vector.tensor_tensor(
                    out=gm[:, :ctot * 2 * OUT].rearrange(
                        "p (c q2 f) -> p c q2 f", q2=2, f=OUT)
                        [:, :, q:q + 1, :],
                    in0=gv[:, :ctot * EL1].rearrange(
                        "p (c q4 f) -> p c q4 f", q4=4, f=OUT)
                        [:, :, q:q + 1, :],
                    in1=maskrd_t[:, c0 * 2:(c0 + ctot) * 2].rearrange(
                        "p (c q2) -> p c q2", q2=2)[:, :, q:q + 1]
                        .unsqueeze(3).broadcast_to([P, ctot, 1, OUT]),
                    op=mybir.AluOpType.mult)
            pag = ppA.tile([P, GB * P], dt.float32, tag="pag")
            for lb in range(nb):
                bw = tail if blo + lb == BLOCKS - 1 else P
                S = build_S_block(blo, lb, nb)
                for cr, sr, cn in block_chunks(lb, nb):
                    for j in range(cn):
                        cc = cr + j
                        nc.tensor.matmul(
                            pag[:2 * OUT, lb * P:lb * P + bw],
                            lhsT=gm[:, cc * 2 * OUT:(cc + 1) * 2 * OUT],
                            rhs=S[:, (sr + j) * P:(sr + j) * P + bw],
                            start=(sr + j == 0),
                            stop=(sr + j == chT - 1))
            aggP1 = sb.tile([2 * OUT, GB * P], dt.float16, tag="aggP1")
            nc.vector.tensor_copy(aggP1[:, :nw], pag[:2 * OUT, :nw])
            ph1 = ppB.tile([HID, GB * P], dt.float32, tag="lin")
            nc.tensor.matmul(ph1[:OUT, :nw], lhsT=i162_t[:],
                             rhs=aggP1[:, :nw], start=True, stop=False)
            nc.tensor.matmul(ph1[:OUT, :nw], lhsT=Wr1T_t[:],
                             rhs=hT_t[:, blo * P:blo * P + nw],
                             start=False, stop=False)
            nc.tensor.matmul(ph1[:OUT, :nw], lhsT=We1T_t[:],
                             rhs=e0T_t[:, blo * P:blo * P + nw],
                             start=False, stop=True)
            oT = sb.tile([OUT, GB * P], dt.float32, tag="o16")
            nc.scalar.activation(oT[:, :nw], ph1[:OUT, :nw],
                                 mybir.ActivationFunctionType.Identity,
                                 bias=b1_t[:])
            for b0_ in range(0, nw, P):
                bw = min(P, nw - b0_)
                ptro = ppC.tile([P, HID], dt.float32, tag="tr")
                nc.tensor.transpose(ptro[:bw, :OUT], oT[:, b0_:b0_ + bw],
                                    ident[:OUT, :OUT])
                osb = sb.tile([P, OUT], dt.float32, tag="osb")
                nc.vector.tensor_copy(osb[:bw, :], ptro[:bw, :OUT])
                nc.sync.dma_start(
                    out_own[blo * P + b0_:blo * P + b0_ + bw, :],
                    osb[:bw, :])

    # --- fix up prepare_only gather completion sems -----------------------
    # Tile assigns each Pool-DMA prep a DMASW{lane} proc and makes data
    # consumers wait on that lane's semaphore, but the semaphore actually
    # baked into the descriptors (fired by SDMA at transfer completion) is
    # the caller's sem= argument. Rewrite each prep's OnUpdate[0] to the
    # scheduler's DMASW lane semaphore so the consumer waits are satisfied
    # by the real DMA completion.
    if PREP_TRIGGER:
        from concourse.tile_scheduler import dmasw_start_idx
        lane_sems = {}
        for bb in nc.m.functions[0].blocks:
            for ins in bb.instructions:
                si = getattr(ins, "sync_info", None)
                if si is None:
                    continue
                for w in si.on_wait:
                    nm = w.ant_name or ""
                    if nm.startswith("DMASW"):
                        lane_sems[int(nm[5:nm.index("_")])] = (w.id, nm)
        for bb in nc.m.functions[0].blocks:
            for ins in bb.instructions:
                if (type(ins).__name__ == "InstDMAGatherAnt"
                        and getattr(ins, "gen_mode", 0) == 1):
                    lane = ins.bass_scheduled_proc - dmasw_start_idx
                    if lane not in lane_sems:
                        continue      # no consumer waits on this lane
                    sid, snm = lane_sems[lane]
                    u = ins.sync_info.on_update[0]
                    assert u.ant_name == "swdge_dma", u.ant_name
                    u.id = sid
                    u.ant_name = snm

    nc.compile()
    return nc


def prep_inputs(cfg, x_feat, x_emb, edge_index,
                Wl0, bl0, Wr0, We0, be0, Wl1, bl1, Wr1, We1, be1):
    """Shard + layout inputs for each core. Pure indexing/layout (plus weight
    transposes and the bias-constant folds); all FLOPs stay on device."""
    sA, sB = cfg["slots_a"], cfg["slots_b"]
    chT = cfg["ch_a"] + cfg["ch_b"]
    spb = sA + sB
    ntok = BLOCKS * spb
    nchunks = BLOCKS * chT

    src, dst = np.asarray(edge_index[0]), np.asarray(edge_index[1])
    in_maps = []
    w_common = dict(
        Wl0T=np.ascontiguousarray(Wl0.T, np.float32),
        Wr0T=np.ascontiguousarray(Wr0.T, np.float32),
        We0Th=np.ascontiguousarray(We0.T * 0.5, np.float32),
        Wr1T=np.ascontiguousarray(Wr1.T).astype(np.float16),
        We1T=np.ascontiguousarray(We1.T).astype(np.float16),
        Wl1T=np.ascontiguousarray(Wl1.T).astype(np.float16),
        b0=np.asarray(bl0, np.float32).reshape(-1, 1),
        be0=np.asarray(be0, np.float32).reshape(-1, 1),
        b1=(np.asarray(bl1, np.float32) + np.asarray(be1, np.float32)
            ).reshape(-1, 1),
        i642=np.tile(np.eye(64, dtype=np.float16), (2, 1)),
        i162=np.tile(np.eye(16, dtype=np.float16), (2, 1)),
        i64f=np.eye(64, dtype=np.float16),
    )
    for c in range(NC_N):
        lo = c * NODES_PER
        m = (dst >= lo) & (dst < lo + NODES_PER)
        s_c, d_c = src[m], dst[m] - lo
        deg = np.bincount(d_c, minlength=NODES_PER)
        rdeg = 1.0 / np.maximum(deg, 1.0)
        blk = d_c >> 7
        strm = (s_c >= SPLIT).astype(np.int64)
        key = blk * 2 + strm
        order = np.argsort(key, kind="stable")
        s_c, d_c, blk, strm, key = (s_c[order], d_c[order], blk[order],
                                    strm[order], key[order])
        cnt = np.bincount(key, minlength=BLOCKS * 2)
        if cnt[0::2].max() > sA or cnt[1::2].max() > sB:
            raise OverflowError(int(cnt[0::2].max()), int(cnt[1::2].max()))
        starts = np.zeros(BLOCKS * 2, np.int64)
        starts[1:] = np.cumsum(cnt)[:-1]
        slot = np.arange(len(d_c)) - starts[key]
        gi = blk // GB
        blo = gi * GB
        nb = np.minimum(blo + GB, BLOCKS) - blo
        lb = blk - blo
        t = blo * spb + np.where(
            strm == 0,
            lb * sA + slot,
            nb * sA + lb * sB + slot)

        srcp = np.zeros(ntok, np.int16)
        srcp[t] = np.where(s_c < SPLIT, s_c >> 1,
                           (s_c - SPLIT) >> 1).astype(np.int16)
        dstrel = np.full((P, nchunks), -1, np.int16)
        dstrel[t % P, t // P] = (d_c & 127).astype(np.int16)
        maskrd = np.zeros((P, nchunks, 2), np.float16)
        maskrd[t % P, t // P, s_c & 1] = rdeg[d_c].astype(np.float16)

        in_maps.append(dict(
            xT=np.ascontiguousarray(
                np.asarray(x_feat[lo:lo + NODES_PER]).T, np.float32),
            xembT=np.ascontiguousarray(
                np.asarray(x_emb[lo:lo + NODES_PER])
                .transpose(1, 2, 0).reshape(2 * EMB, NODES_PER), np.float32),
            srcp=np.ascontiguousarray(np.tile(srcp.reshape(-1, 16).T, (8, 1))),
            dstrel=np.ascontiguousarray(dstrel),
            maskrd=np.ascontiguousarray(maskrd.reshape(P, -1)),
            **w_common,
        ))
    return in_maps


def kernel(**inputs):
    src_dst = np.asarray(inputs["edge_index"])
    src, dst = src_dst[0], src_dst[1]
    mxA = mxB = 0
    for c in range(NC_N):
        lo = c * NODES_PER
        m = (dst >= lo) & (dst < lo + NODES_PER)
        s_c, d_c = src[m], dst[m] - lo
        key = (d_c >> 7) * 2 + (s_c >= SPLIT)
        cnt = np.bincount(key, minlength=BLOCKS * 2)
        mxA = max(mxA, int(cnt[0::2].max()))
        mxB = max(mxB, int(cnt[1::2].max()))
    slots_a = max(((mxA + P - 1) // P) * P, P)
    slots_b = max(((mxB + P - 1) // P) * P, P)
    cfg = default_cfg(slots_a, slots_b)
    nc = build_nc(cfg)
    in_maps = prep_inputs(cfg, **inputs)
    res = run_bass_kernel_spmd(nc, in_maps, list(range(NC_N)))
    kernel.last_res = res
    return np.concatenate([res.results[c]["out_own"] for c in range(NC_N)],
                          axis=0)


# revision 35
# speedup vs baseline: 1.0880x; 1.0466x over previous
"""GraphSAGE 2-layer forward on 8 Trainium2 NeuronCores (Bass/Tile).

Strategy (graph/data parallel, per sharding hint):
  - Destination nodes partitioned across 8 cores (12500 each); edges bucketed
    to the core owning their dst, grouped into 128-dst-node blocks, and blocks
    into groups of gb=4 (512 dsts) for coarse-grained pipelining.
  - Per-edge payloads are fetched with dma_gather in prepare_only mode +
    trigger_dma, so the Pool engine only spends descriptor-generation time and
    the SDMA transfers overlap with desc-gen of the next group and all compute
    (the v1 kernel's blocking gathers serialized the whole kernel).
  - 256B tokens (the SWDGE minimum) with 2 packed nodes per token:
      L0 table y0 = x @ Wl0.T as [50000, 128] fp16 (node pairs),
      L1 table y1 = h @ Wl1.T as [50000, 64] fp32 (2x16 values + 32 pad).
    int16 gather indices only span 32768 rows, so edges are split into a
    lo stream (src < 65536) and a hi stream with a rebased table AP.
  - A per-token mask*1/deg vector zeroes the wrong packed node and folds in
    the mean normalization; segment-sum over dst is one-hot matmul on PE:
    S[k, j] = (dst_rel[k] == j), psum[pair-feat, dst] += g.T @ S, pairs folded
    with a stacked-identity matmul.
  - Weights replicated; dense matmuls keep features on partitions; e0 is
    re-added per group via an identity matmul so everything lands in psum.
"""
import numpy as np
from contextlib import ExitStack

from concourse import bass, bacc, mybir, tile
from concourse.bass_utils import run_bass_kernel_spmd

dt = mybir.dt
PREP_TRIGGER = False   # prepare_only+trigger corrupts data on HW; keep off
KMAX = 2048            # max idxs per dma_gather call (SWDGE ring capacity)
SINGLE_PACKET = False  # stream packets per engine instead of one big packet

# problem constants (hardcoded per harness contract)
N = 100000
FEAT = 128
EMB = 64
HID = 64
OUT = 16
NC_N = 8
NODES_PER = N // NC_N          # 12500
P = 128
BLOCKS = (NODES_PER + P - 1) // P   # 98
GB = 4                          # blocks per group
NGROUPS = (BLOCKS + GB - 1) // GB   # 25 (24x4 + 1x2)
SPLIT = 65536                   # lo/hi src stream split (int16 idx range)


def default_cfg(slots_a, slots_b):
    return dict(slots_a=slots_a, slots_b=slots_b,
                ch_a=slots_a // P, ch_b=slots_b // P)


def build_nc(cfg):
    sA, sB = cfg["slots_a"], cfg["slots_b"]
    chA, chB = cfg["ch_a"], cfg["ch_b"]
    chT = chA + chB
    spb = sA + sB                    # slots (tokens) per block
    ntok = BLOCKS * spb              # gather tokens per core
    nchunks = BLOCKS * chT
    tail = NODES_PER - (BLOCKS - 1) * P

    nc = bacc.Bacc("TRN2", target_bir_lowering=False, debug=False,
                   num_devices=NC_N, dynamic_dma_scratch_size=32768,
                   detect_race_conditions=False)

    def din(name, shape, d):
        return nc.dram_tensor(name, shape, d, kind="ExternalInput").ap()

    # --- inputs (per core) ---
    xT = din("xT", [FEAT, NODES_PER], dt.float32)
    xembT = din("xembT", [2 * EMB, NODES_PER], dt.float32)
    srcp = din("srcp", [P, ntok // 16], dt.int16)
    dstrel = din("dstrel", [P, nchunks], dt.int16)
    maskrd = din("maskrd", [P, nchunks * 2], dt.float16)
    Wl0T = din("Wl0T", [FEAT, HID], dt.float32)
    Wr0T = din("Wr0T", [FEAT, HID], dt.float32)
    We0Th = din("We0Th", [EMB, HID], dt.float32)     # We0.T * 0.5
    Wr1T = din("Wr1T", [HID, OUT], dt.float16)
    We1T = din("We1T", [HID, OUT], dt.float16)
    Wl1T = din("Wl1T", [HID, OUT], dt.float16)
    b0 = din("b0", [HID, 1], dt.float32)             # bl0
    be0 = din("be0", [HID, 1], dt.float32)
    b1 = din("b1", [OUT, 1], dt.float32)             # bl1 + be1
    i642 = din("i642", [P, HID], dt.float16)         # vstack(I64, I64)
    i162 = din("i162", [2 * OUT, OUT], dt.float16)   # vstack(I16, I16)
    i64f = din("i64f", [EMB, HID], dt.float16)       # I64 fp16
    out_own = nc.dram_tensor("out_own", [NODES_PER, OUT], dt.float32,
                             kind="ExternalOutput").ap()

    EL0 = 2 * HID                  # 128 fp16 = 256B per token (2 nodes)
    EL1 = 4 * OUT                  # 64 fp32 = 256B per token (2 nodes + pad)
    gcolsmax = GB * chT            # chunk capacity of the per-group g tile

    with tile.TileContext(nc) as tc, ExitStack() as ctx:
        const = ctx.enter_context(tc.tile_pool(name="const", bufs=1))
        sb = ctx.enter_context(tc.tile_pool(name="sb", bufs=2))
        sb3 = ctx.enter_context(tc.tile_pool(name="sb3", bufs=2))
        sbg = ctx.enter_context(tc.tile_pool(name="sbg", bufs=1))
        ppA = ctx.enter_context(tc.tile_pool(name="ppA", bufs=2, space="PSUM"))
        ppB = ctx.enter_context(tc.tile_pool(name="ppB", bufs=2, space="PSUM"))
        ppC = ctx.enter_context(tc.tile_pool(name="ppC", bufs=2, space="PSUM"))
        dram = ctx.enter_context(tc.tile_pool(name="dram", bufs=1, space="DRAM"))

        # --- resident constants ---
        srcp_t = const.tile([P, ntok // 16], dt.int16)
        nc.sync.dma_start(srcp_t[:], srcp[:])
        dstrel_t = const.tile([P, nchunks], dt.int16)
        nc.sync.dma_start(dstrel_t[:], dstrel[:])
        maskrd_t = const.tile([P, nchunks * 2], dt.float16)
        nc.sync.dma_start(maskrd_t[:], maskrd[:])
        iota_t = const.tile([P, max(chA, chB) * P], dt.int16)
        nc.gpsimd.iota(iota_t[:], pattern=[[0, max(chA, chB)], [1, P]],
                       base=0, channel_multiplier=0)
        Wl0T_t = const.tile([FEAT, HID], dt.float32)
        nc.sync.dma_start(Wl0T_t[:], Wl0T[:])
        Wr0T_t = const.tile([FEAT, HID], dt.float32)
        nc.sync.dma_start(Wr0T_t[:], Wr0T[:])
        We0Th_t = const.tile([EMB, HID], dt.float32)
        nc.sync.dma_start(We0Th_t[:], We0Th[:])
        Wr1T_t = const.tile([HID, OUT], dt.float16)
        nc.sync.dma_start(Wr1T_t[:], Wr1T[:])
        We1T_t = const.tile([HID, OUT], dt.float16)
        nc.sync.dma_start(We1T_t[:], We1T[:])
        Wl1T_t = const.tile([HID, OUT], dt.float16)
        nc.sync.dma_start(Wl1T_t[:], Wl1T[:])
        b0_t = const.tile([HID, 1], dt.float32)
        nc.sync.dma_start(b0_t[:], b0[:])
        be0_t = const.tile([HID, 1], dt.float32)
        nc.sync.dma_start(be0_t[:], be0[:])
        b1_t = const.tile([OUT, 1], dt.float32)
        nc.sync.dma_start(b1_t[:], b1[:])
        i642_t = const.tile([P, HID], dt.float16)
        nc.sync.dma_start(i642_t[:], i642[:])
        i162_t = const.tile([2 * OUT, OUT], dt.float16)
        nc.sync.dma_start(i162_t[:], i162[:])
        i64f_t = const.tile([EMB, HID], dt.float16)
        nc.sync.dma_start(i64f_t[:], i64f[:])
        ident = const.tile([P, P], dt.float32)
        from concourse.masks import make_identity
        make_identity(nc, ident[:])
        zz32 = const.tile([P, 2 * OUT], dt.float32)
        nc.gpsimd.memset(zz32[:], 0.0)

        # persistent activations (features on partitions)
        e0T_t = const.tile([EMB, NODES_PER], dt.float16)
        hT_t = const.tile([HID, NODES_PER], dt.float16)

        dma_sem = nc.alloc_semaphore("swdge_dma")

        # =========== Phase 2: y0 = x @ Wl0.T (own nodes, pair-major fp16) ====
        y0_own = dram.tile([NODES_PER // 2, EL0], dt.float16)
        G = 512
        for g0 in range(0, NODES_PER, G):
            g1 = min(g0 + G, NODES_PER)
            w = g1 - g0
            xg = sb.tile([FEAT, G], dt.float32, tag="xg")
            nc.sync.dma_start(xg[:, :w], xT[:, g0:g1])
            py0 = ppB.tile([HID, G], dt.float32, tag="lin")
            nc.tensor.matmul(py0[:, :w], lhsT=Wl0T_t[:], rhs=xg[:, :w],
                             start=True, stop=True)
            y0g = sb.tile([HID, G], dt.float32, tag="y0g")
            nc.vector.tensor_copy(y0g[:, :w], py0[:, :w])
            for b0_ in range(0, w, P):
                b1_ = min(b0_ + P, w)
                bw = b1_ - b0_
                ptr = ppC.tile([P, HID], dt.float32, tag="tr")
                nc.tensor.transpose(ptr[:bw, :], y0g[:, b0_:b1_],
                                    ident[:HID, :HID])
                str_ = sb.tile([P, HID], dt.float16, tag="str")
                nc.vector.tensor_copy(str_[:bw, :], ptr[:bw, :])
                r0 = (g0 + b0_) // 2
                nc.sync.dma_start(
                    y0_own[r0:r0 + bw // 2, :].rearrange(
                        "r (a f) -> r a f", a=2),
                    str_[:bw, :])

        # =========== Phase 3: AllGather y0 -> table [50000, 128] f16 =========
        y0_full = dram.tile([N // 2, EL0], dt.float16, addr_space="Shared")
        nc.gpsimd.collective_compute(
            "AllGather", mybir.AluOpType.bypass,
            replica_groups=[list(range(NC_N))],
            ins=[y0_own[:]], outs=[y0_full[:]],
        )
        y0_lo = y0_full[0:SPLIT // 2, :]
        y0_hi = y0_full[SPLIT // 2:N // 2, :]

        # =========== Phase 1: e0T = (emb0+emb1) @ (0.5 We0.T) + be0 ==========
        # (after the AllGather kick so it overlaps the collective)
        for g0 in range(0, NODES_PER, G):
            g1 = min(g0 + G, NODES_PER)
            w = g1 - g0
            xe0 = sb.tile([EMB, G], dt.float32, tag="xe0")
            nc.sync.dma_start(xe0[:, :w], xembT[:EMB, g0:g1])
            xe1 = sb.tile([EMB, G], dt.float32, tag="xe1")
            nc.sync.dma_start(xe1[:, :w], xembT[EMB:, g0:g1])
            pe0 = ppB.tile([HID, G], dt.float32, tag="lin")
            nc.tensor.matmul(pe0[:, :w], lhsT=We0Th_t[:], rhs=xe0[:, :w],
                             start=True, stop=False)
            nc.tensor.matmul(pe0[:, :w], lhsT=We0Th_t[:], rhs=xe1[:, :w],
                             start=False, stop=True)
            nc.scalar.activation(e0T_t[:, g0:g1], pe0[:, :w],
                                 mybir.ActivationFunctionType.Identity,
                                 bias=be0_t[:])

        # =========== Phase 4: L0 aggregation + h assembly + y1 ===============
        y1_own = dram.tile([NODES_PER // 2, EL1], dt.float32)

        def groups():
            for gi in range(NGROUPS):
                blo = gi * GB
                bhi = min(blo + GB, BLOCKS)
                yield gi, blo, bhi, bhi - blo

        def gather_group(blo, nb, tab_lo, tab_hi, elem):
            """Issue lo+hi stream preps + trigger for blocks [blo, blo+nb)."""
            ctot = nb * chT
            g = sb3.tile([P, gcolsmax * EL0], dt.float16, tag="g")
            gv = g[:] if elem == EL0 else g[:].bitcast(dt.float32)
            t0 = blo * spb
            ga = nb * sA                      # lo-stream tokens in group
            gt = nb * spb
            for base, n, tab, cof in ((t0, ga, tab_lo, 0),
                                      (t0 + ga, nb * sB, tab_hi,
                                       nb * chA)):
                for k0 in range(0, n, KMAX):
                    k1 = min(k0 + KMAX, n)
                    kw = dict(prepare_only=True, sem=dma_sem) \
                        if PREP_TRIGGER else {}
                    kw["single_packet"] = SINGLE_PACKET
                    nc.gpsimd.dma_gather(
                        out_ap=gv[:, (cof * P + k0) * elem // P:
                                  (cof * P + k1) * elem // P].rearrange(
                            "p (c e) -> p c e", e=elem),
                        in_ap=tab,
                        idxs_ap=srcp_t[:, (base + k0) // 16:
                                       (base + k1) // 16],
                        num_idxs=k1 - k0, num_idxs_reg=k1 - k0,
                        elem_size=elem, **kw,
                    )
                    if PREP_TRIGGER:
                        nc.gpsimd.trigger_dma(count=None)
            return g, gv

        def build_S_block(blo, lb, nb):
            """One-hot S for block lb of the group: A chunks then B chunks."""
            c0 = blo * chT
            S = sb3.tile([P, chT * P], dt.float16, tag="S")
            for (sr, cn, cglob) in (
                (0, chA, c0 + lb * chA),
                (chA, chB, c0 + nb * chA + lb * chB),
            ):
                dr = dstrel_t[:, cglob:cglob + cn]
                nc.vector.tensor_tensor(
                    out=S[:, sr * P:(sr + cn) * P].rearrange(
                        "p (c q) -> p c q", q=P),
                    in0=iota_t[:, :cn * P].rearrange(
                        "p (c q) -> p c q", q=P),
                    in1=dr.unsqueeze(2).broadcast_to([P, cn, P]),
                    op=mybir.AluOpType.is_equal)
            return S

        def block_chunks(lb, nb):
            """(g-col, S-col, n) chunk runs of block lb in the group tile."""
            return ((lb * chA, 0, chA), (nb * chA + lb * chB, chA, chB))

        for gi, blo, bhi, nb in groups():
            ctot = nb * chT
            c0 = blo * chT
            nw = (bhi - blo - 1) * P + (tail if bhi == BLOCKS else P)
            g, _ = gather_group(blo, nb, y0_lo, y0_hi, EL0)
            # mask*1/deg zeroes the wrong node of each pair
            g4 = g[:, :ctot * EL0].rearrange("p (q f) -> p q f", f=HID)
            mr = maskrd_t[:, c0 * 2:(c0 + ctot) * 2]
            nc.vector.tensor_tensor(
                out=g4, in0=g4,
                in1=mr.unsqueeze(2).broadcast_to([P, ctot * 2, HID]),
                op=mybir.AluOpType.mult)
            pag = ppA.tile([P, GB * P], dt.float32, tag="pag")
            for lb in range(nb):
                bw = tail if blo + lb == BLOCKS - 1 else P
                S = build_S_block(blo, lb, nb)
                for cr, sr, cn in block_chunks(lb, nb):
                    for j in range(cn):
                        nc.tensor.matmul(
                            pag[:, lb * P:lb * P + bw],
                            lhsT=g[:, (cr + j) * EL0:(cr + j + 1) * EL0],
                            rhs=S[:, (sr + j) * P:(sr + j) * P + bw],
                            start=(sr + j == 0),
                            stop=(sr + j == chT - 1))
            aggP = sb.tile([P, GB * P], dt.float16, tag="aggP")
            nc.vector.tensor_copy(aggP[:, :nw], pag[:, :nw])
            xb = sb.tile([FEAT, GB * P], dt.float32, tag="xg")
            nc.sync.dma_start(xb[:, :nw], xT[:, blo * P:blo * P + nw])
            ph = ppB.tile([HID, GB * P], dt.float32, tag="lin")
            nc.tensor.matmul(ph[:, :nw], lhsT=Wr0T_t[:], rhs=xb[:, :nw],
                             start=True, stop=False)
            nc.tensor.matmul(ph[:, :nw], lhsT=i64f_t[:],
                             rhs=e0T_t[:, blo * P:blo * P + nw],
                             start=False, stop=False)
            nc.tensor.matmul(ph[:, :nw], lhsT=i642_t[:], rhs=aggP[:, :nw],
                             start=False, stop=True)
            # h = relu(agg + xWr0 + e0 + bl0)
            nc.scalar.activation(hT_t[:, blo * P:blo * P + nw], ph[:, :nw],
                                 mybir.ActivationFunctionType.Relu,
                                 bias=b0_t[:])
            # y1 = h @ Wl1.T -> pair-major fp32 (+zero pad cols)
            py1 = ppB.tile([HID, GB * P], dt.float32, tag="lin")
            nc.tensor.matmul(py1[:OUT, :nw], lhsT=Wl1T_t[:],
                             rhs=hT_t[:, blo * P:blo * P + nw],
                             start=True, stop=True)
            y1sb = sb.tile([OUT, GB * P], dt.float32, tag="o16")
            nc.vector.tensor_copy(y1sb[:, :nw], py1[:OUT, :nw])
            for b0_ in range(0, nw, P):
                bw = min(P, nw - b0_)
                ptr1 = ppC.tile([P, HID], dt.float32, tag="tr")
                nc.tensor.transpose(ptr1[:bw, :OUT], y1sb[:, b0_:b0_ + bw],
                                    ident[:OUT, :OUT])
                y1tr = sb.tile([P, OUT], dt.float32, tag="y1tr")
                nc.vector.tensor_copy(y1tr[:bw, :], ptr1[:bw, :OUT])
                r0 = (blo * P + b0_) // 2
                nc.sync.dma_start(
                    y1_own[r0:r0 + bw // 2, 0:2 * OUT].rearrange(
                        "r (a f) -> r a f", a=2),
                    y1tr[:bw, :])
                nc.sync.dma_start(
                    y1_own[r0:r0 + bw // 2, 2 * OUT:4 * OUT].rearrange(
                        "r (a f) -> r a f", a=2),
                    zz32[:bw, :OUT])

        # =========== Phase 5: AllGather y1 ===================================
        y1_full = dram.tile([N // 2, EL1], dt.float32, addr_space="Shared")
        nc.gpsimd.collective_compute(
            "AllGather", mybir.AluOpType.bypass,
            replica_groups=[list(range(NC_N))],
            ins=[y1_own[:]], outs=[y1_full[:]],
        )
        y1_lo = y1_full[0:SPLIT // 2, :]
        y1_hi = y1_full[SPLIT // 2:N // 2, :]

        # =========== Phase 6: L1 aggregation + output ========================
        for gi, blo, bhi, nb in groups():
            ctot = nb * chT
            c0 = blo * chT
            nw = (bhi - blo - 1) * P + (tail if bhi == BLOCKS else P)
            g, gv = gather_group(blo, nb, y1_lo, y1_hi, EL1)
            # masked compaction: keep the 2 real 16-wide rows of each token
            gm = sbg.tile([P, gcolsmax * 2 * OUT], dt.float16, tag="gm")
            for q in range(2):
                nc.vector.tensor_tensor(
                    out=gm[:, :ctot * 2 * OUT].rearrange(
                        "p (c q2 f) -> p c q2 f", q2=2, f=OUT)
                        [:, :, q:q + 1, :],
                    in0=gv[:, :ctot * EL1].rearrange(
                        "p (c q4 f) -> p c q4 f", q4=4, f=OUT)
                        [:, :, q:q + 1, :],
                    in1=maskrd_t[:, c0 * 2:(c0 + ctot) * 2].rearrange(
                        "p (c q2) -> p c q2", q2=2)[:, :, q:q + 1]
                        .unsqueeze(3).broadcast_to([P, ctot, 1, OUT]),
                    op=mybir.AluOpType.mult)
            pag = ppA.tile([P, GB * P], dt.float32, tag="pag")
            for lb in range(nb):
                bw = tail if blo + lb == BLOCKS - 1 else P
                S = build_S_block(blo, lb, nb)
                for cr, sr, cn in block_chunks(lb, nb):
                    for j in range(cn):
                        cc = cr + j
                        nc.tensor.matmul(
                            pag[:2 * OUT, lb * P:lb * P + bw],
                            lhsT=gm[:, cc * 2 * OUT:(cc + 1) * 2 * OUT],
                            rhs=S[:, (sr + j) * P:(sr + j) * P + bw],
                            start=(sr + j == 0),
                            stop=(sr + j == chT - 1))
            aggP1 = sb.tile([2 * OUT, GB * P], dt.float16, tag="aggP1")
            nc.vector.tensor_copy(aggP1[:, :nw], pag[:2 * OUT, :nw])
            ph1 = ppB.tile([HID, GB * P], dt.float32, tag="lin")
            nc.tensor.matmul(ph1[:OUT, :nw], lhsT=i162_t[:],
                             rhs=aggP1[:, :nw], start=True, stop=False)
            nc.tensor.matmul(ph1[:OUT, :nw], lhsT=Wr1T_t[:],
                             rhs=hT_t[:, blo * P:blo * P + nw],
                             start=False, stop=False)
            nc.tensor.matmul(ph1[:OUT, :nw], lhsT=We1T_t[:],
                             rhs=e0T_t[:, blo * P:blo * P + nw],
                             start=False, stop=True)
            oT = sb.tile([OUT, GB * P], dt.float32, tag="o16")
            nc.scalar.activation(oT[:, :nw], ph1[:OUT, :nw],
                                 mybir.ActivationFunctionType.Identity,
                                 bias=b1_t[:])
            for b0_ in range(0, nw, P):
                bw = min(P, nw - b0_)
                ptro = ppC.tile([P, HID], dt.float32, tag="tr")
                nc.tensor.transpose(ptro[:bw, :OUT], oT[:, b0_:b0_ + bw],
                                    ident[:OUT, :OUT])
                osb = sb.tile([P, OUT], dt.float32, tag="osb")
                nc.vector.tensor_copy(osb[:bw, :], ptro[:bw, :OUT])
                nc.sync.dma_start(
                    out_own[blo * P + b0_:blo * P + b0_ + bw, :],
                    osb[:bw, :])

    # --- fix up prepare_only gather completion sems -----------------------
    # Tile assigns each Pool-DMA prep a DMASW{lane} proc and makes data
    # consumers wait on that lane's semaphore, but the semaphore actually
    # baked into the descriptors (fired by SDMA at transfer completion) is
    # the caller's sem= argument. Rewrite each prep's OnUpdate[0] to the
    # scheduler's DMASW lane semaphore so the consumer waits are satisfied
    # by the real DMA completion.
    if PREP_TRIGGER:
        from concourse.tile_scheduler import dmasw_start_idx
        lane_sems = {}
        for bb in nc.m.functions[0].blocks:
            for ins in bb.instructions:
                si = getattr(ins, "sync_info", None)
                if si is None:
                    continue
                for w in si.on_wait:
                    nm = w.ant_name or ""
                    if nm.startswith("DMASW"):
                        lane_sems[int(nm[5:nm.index("_")])] = (w.id, nm)
        for bb in nc.m.functions[0].blocks:
            for ins in bb.instructions:
                if (type(ins).__name__ == "InstDMAGatherAnt"
                        and getattr(ins, "gen_mode", 0) == 1):
                    lane = ins.bass_scheduled_proc - dmasw_start_idx
                    if lane not in lane_sems:
                        continue      # no consumer waits on this lane
                    sid, snm = lane_sems[lane]
                    u = ins.sync_info.on_update[0]
                    assert u.ant_name == "swdge_dma", u.ant_name
                    u.id = sid
                    u.ant_name = snm

    nc.compile()
    return nc


def prep_inputs(cfg, x_feat, x_emb, edge_index,
                Wl0, bl0, Wr0, We0, be0, Wl1, bl1, Wr1, We1, be1):
    """Shard + layout inputs for each core. Pure indexing/layout (plus weight
    transposes and the bias-constant folds); all FLOPs stay on device."""
    sA, sB = cfg["slots_a"], cfg["slots_b"]
    chT = cfg["ch_a"] + cfg["ch_b"]
    spb = sA + sB
    ntok = BLOCKS * spb
    nchunks = BLOCKS * chT

    src, dst = np.asarray(edge_index[0]), np.asarray(edge_index[1])
    in_maps = []
    w_common = dict(
        Wl0T=np.ascontiguousarray(Wl0.T, np.float32),
        Wr0T=np.ascontiguousarray(Wr0.T, np.float32),
        We0Th=np.ascontiguousarray(We0.T * 0.5, np.float32),
        Wr1T=np.ascontiguousarray(Wr1.T).astype(np.float16),
        We1T=np.ascontiguousarray(We1.T).astype(np.float16),
        Wl1T=np.ascontiguousarray(Wl1.T).astype(np.float16),
        b0=np.asarray(bl0, np.float32).reshape(-1, 1),
        be0=np.asarray(be0, np.float32).reshape(-1, 1),
        b1=(np.asarray(bl1, np.float32) + np.asarray(be1, np.float32)
            ).reshape(-1, 1),
        i642=np.tile(np.eye(64, dtype=np.float16), (2, 1)),
        i162=np.tile(np.eye(16, dtype=np.float16), (2, 1)),
        i64f=np.eye(64, dtype=np.float16),
    )
    for c in range(NC_N):
        lo = c * NODES_PER
        m = (dst >= lo) & (dst < lo + NODES_PER)
        s_c, d_c = src[m], dst[m] - lo
        deg = np.bincount(d_c, minlength=NODES_PER)
        rdeg = 1.0 / np.maximum(deg, 1.0)
        blk = d_c >> 7
        strm = (s_c >= SPLIT).astype(np.int64)
        key = blk * 2 + strm
        order = np.argsort(key, kind="stable")
        s_c, d_c, blk, strm, key = (s_c[order], d_c[order], blk[order],
                                    strm[order], key[order])
        cnt = np.bincount(key, minlength=BLOCKS * 2)
        if cnt[0::2].max() > sA or cnt[1::2].max() > sB:
            raise OverflowError(int(cnt[0::2].max()), int(cnt[1::2].max()))
        starts = np.zeros(BLOCKS * 2, np.int64)
        starts[1:] = np.cumsum(cnt)[:-1]
        slot = np.arange(len(d_c)) - starts[key]
        gi = blk // GB
        blo = gi * GB
        nb = np.minimum(blo + GB, BLOCKS) - blo
        lb = blk - blo
        t = blo * spb + np.where(
            strm == 0,
            lb * sA + slot,
            nb * sA + lb * sB + slot)

        srcp = np.zeros(ntok, np.int16)
        srcp[t] = np.where(s_c < SPLIT, s_c >> 1,
                           (s_c - SPLIT) >> 1).astype(np.int16)
        dstrel = np.full((P, nchunks), -1, np.int16)
        dstrel[t % P, t // P] = (d_c & 127).astype(np.int16)
        maskrd = np.zeros((P, nchunks, 2), np.float16)
        maskrd[t % P, t // P, s_c & 1] = rdeg[d_c].astype(np.float16)

        in_maps.append(dict(
            xT=np.ascontiguousarray(
                np.asarray(x_feat[lo:lo + NODES_PER]).T, np.float32),
            xembT=np.ascontiguousarray(
                np.asarray(x_emb[lo:lo + NODES_PER])
                .transpose(1, 2, 0).reshape(2 * EMB, NODES_PER), np.float32),
            srcp=np.ascontiguousarray(np.tile(srcp.reshape(-1, 16).T, (8, 1))),
            dstrel=np.ascontiguousarray(dstrel),
            maskrd=np.ascontiguousarray(maskrd.reshape(P, -1)),
            **w_common,
        ))
    return in_maps


def kernel(**inputs):
    src_dst = np.asarray(inputs["edge_index"])
    src, dst = src_dst[0], src_dst[1]
    mxA = mxB = 0
    for c in range(NC_N):
        lo = c * NODES_PER
        m = (dst >= lo) & (dst < lo + NODES_PER)
        s_c, d_c = src[m], dst[m] - lo
        key = (d_c >> 7) * 2 + (s_c >= SPLIT)
        cnt = np.bincount(key, minlength=BLOCKS * 2)
        mxA = max(mxA, int(cnt[0::2].max()))
        mxB = max(mxB, int(cnt[1::2].max()))
    slots_a = max(((mxA + P - 1) // P) * P, P)
    slots_b = max(((mxB + P - 1) // P) * P, P)
    cfg = default_cfg(slots_a, slots_b)
    nc = build_nc(cfg)
    in_maps = prep_inputs(cfg, **inputs)
    res = run_bass_kernel_spmd(nc, in_maps, list(range(NC_N)))
    kernel.last_res = res
    return np.concatenate([res.results[c]["out_own"] for c in range(NC_N)],
                          axis=0)
